# revision 12
# baseline (speedup 1.0000x reference)
"""Bass/Trainium2 kernel for nn_DecorrelationGradient.

Reference computation (KAPPA = 0.5):
    out = (1-k)*(gram - diag_ms) + k*(diag_ms - 1)
        = 0.5 * (X^T X / N) - 0.5          (diag terms cancel algebraically)

with X = x.reshape(N, d), N = 8*2048 = 16384, d = 768.

Strategy (data-parallel over the sample axis, 8 cores):
  - core c gets x[c] : [2048, 768] f32
  - per-core pipeline: HWDGE f32 loads (both rings) -> DVE cast to fp8 e4m3
    -> PE computes the upper-triangle blocks of the partial Gram x_c^T x_c
    with fp8 DoubleRow matmuls (256-sample contraction per matmul, ~2x the
    bf16 column rate; fp32 PSUM accumulation over 8 sample-groups).  The
    load stream runs at the per-core HBM limit and paces the kernel; the
    fp8 PE work hides entirely under it.
  - no on-device scale/bias: each core outputs its RAW partial Gram
    triangle packed [128, 2688] in bf16; the host gather sums the 8
    partials in f32 and applies out = sum * (0.5/N) - 0.5.  (fp8 inputs +
    bf16 partial output keep the overall rel err ~2-3e-4, well inside the
    1e-3 gate; a device-side ReduceScatter measures ~2.5x slower here
    because the collective serializes behind a cross-core launch-skew
    barrier plus ncfw trigger latency.)
"""

import numpy as np

import concourse.bacc as bacc
import concourse.bass as bass  # noqa: F401
import concourse.tile as tile
from concourse import mybir
from concourse.bass_utils import run_bass_kernel_spmd

P = 128
D = 768
NSHARD = 2048          # samples per core
KT = NSHARD // P       # 16 DMA tiles of 128 samples
NG = KT // 2           # 8 DoubleRow groups of 256 samples
NB = D // P            # 6 row/col blocks
NCORES = 8
NTOT = NCORES * NSHARD
SCALE = 0.5 / NTOT     # applied on host
BIAS = -0.5            # applied on host

# packed upper-triangle blocks (i, j) with j >= i, row-major in i
TRI_BLOCKS = [(i, j) for i in range(NB) for j in range(i, NB)]
NTRI = len(TRI_BLOCKS)          # 21
TRI_W = NTRI * P                # 2688 packed columns

# packed column range of row-block i
OFFS = []
_o = 0
for _i in range(NB):
    OFFS.append((_o, _o + (NB - _i) * P))
    _o = OFFS[-1][1]

# per-group matmul chunks (i, s0, s1, last_of_i), big blocks first so the
# widest copy-outs can start earliest in the final group, and so the
# LDWEIGHTS stream (one per block, overlapped via the background weight
# buffer) keeps pace with the matmul stream
CHUNKS = []
for _i in range(NB):
    w = D - P * _i
    if w > 512:
        CHUNKS.append((_i, 0, 512, False))
        CHUNKS.append((_i, 512, w, True))
    else:
        CHUNKS.append((_i, 0, w, True))


def _copy_out(nc, tri, pss, i):
    """Plain PSUM->SBUF bf16 copy of row-block i (scale/bias is host-side).
    Blocks alternate between the ACT and DVE engines so consecutive blocks
    copy in parallel as their accumulators finish (GPSIMD has no PSUM port)."""
    o0, o1 = OFFS[i]
    if i % 2 == 0:
        nc.scalar.activation(
            out=tri[:, o0:o1],
            in_=pss[i][:, 0 : o1 - o0],
            func=mybir.ActivationFunctionType.Copy,
        )
    else:
        nc.vector.tensor_copy(out=tri[:, o0:o1], in_=pss[i][:, 0 : o1 - o0])


def _build():
    nc = bacc.Bacc(num_devices=NCORES)

    x_sh = nc.dram_tensor(
        "x_shard", [NSHARD, D], mybir.dt.float32, kind="ExternalInput"
    )
    out_sh = nc.dram_tensor(
        "out_shard", [P, TRI_W], mybir.dt.bfloat16, kind="ExternalOutput"
    )

    f32 = mybir.dt.float32
    bf16 = mybir.dt.bfloat16
    fp8 = mybir.dt.float8e4

    with tile.TileContext(nc) as tc:
        with (
            tc.tile_pool(name="xp", bufs=KT) as xpool,
            tc.tile_pool(name="gp", bufs=NG) as gpool,
            tc.tile_pool(name="ps", bufs=1, space="PSUM") as pspool,
            tc.tile_pool(name="acc", bufs=1) as accpool,
        ):
            # HAM warmup tile: first thing in program order so the junk
            # matmuls fill the PE pipe while the first x tiles stream in
            warm = gpool.tile([P, 512], bf16, tag="warm", name="warm")
            nc.gpsimd.memset(warm[:], 0.0)

            # pipeline per 128-sample tile: HWDGE f32 DMA -> DVE cast to
            # fp8 plane (k%2) of group k//2.  loads alternate between the
            # two physical HWDGE rings (SP / ACT)
            grps = []
            for g in range(NG):
                grps.append(gpool.tile([P, 2, D], fp8, tag="gx", name=f"gx{g}"))
            for k in range(KT):
                stage = xpool.tile([P, D], f32, tag="xs", name=f"xs{k}")
                if k < KT - 2:
                    dma_eng = nc.sync if k % 2 == 0 else nc.scalar
                    dma_eng.dma_start(
                        out=stage[:], in_=x_sh[k * P : (k + 1) * P, :]
                    )
                    nc.vector.tensor_copy(
                        out=grps[k // 2][:, k % 2, :], in_=stage[:]
                    )
                else:
                    # last pair: split each tile into column halves on both
                    # rings and cast per half, so only ~half a cast remains
                    # after the final DMA lands (shorter serial tail)
                    h = D // 2
                    nc.sync.dma_start(
                        out=stage[:, 0:h], in_=x_sh[k * P : (k + 1) * P, 0:h]
                    )
                    nc.scalar.dma_start(
                        out=stage[:, h:D], in_=x_sh[k * P : (k + 1) * P, h:D]
                    )
                    nc.vector.tensor_copy(
                        out=grps[k // 2][:, k % 2, 0:h], in_=stage[:, 0:h]
                    )
                    nc.vector.tensor_copy(
                        out=grps[k // 2][:, k % 2, h:D], in_=stage[:, h:D]
                    )

            tri = accpool.tile([P, TRI_W], bf16)  # packed raw partial triangle

            # psum accumulators, one per row-block; exactly 8 PSUM banks.
            # row-block i covers G[i-block, j-blocks j>=i] = cols 128*i..768.
            # ps0 is padded to its full 2-bank allocation; the padding columns
            # [768:1024] are a scratch target for the HAM-pinning junk matmuls
            # (per-element has_written bits keep them out of the real data).
            ps0_full = pspool.tile([P, 1024], f32, tag="ps0", name="ps0")
            pss = [ps0_full]
            for i in range(1, NB):
                pss.append(
                    pspool.tile([P, D - P * i], f32, tag=f"ps{i}", name=f"ps{i}")
                )

            # HAM warmup: junk matmuls on the zeroed tile keep the PE busy
            # until the first real tile lands.  The fp8 groups leave the PE
            # at ~55% duty behind the load stream, which is not dense enough
            # to flip the HAM activity window on its own — so run ~3.8us of
            # dense junk up front to un-throttle the clock before group 0.
            # Junk goes to pss[0]; the real g=0 matmul start=True resets it.
            for w in range(9):
                nc.tensor.matmul(
                    pss[0][:, 0:512],
                    lhsT=warm[:, 0:P],
                    rhs=warm[:],
                    start=True,
                    stop=True,
                )

            # group-outer / chunk-inner: each 256-sample fp8 group is fully
            # consumed right after its two DMA+cast tiles land, so the PE
            # overlaps the load stream.  DoubleRow: lhsT [128,2,128] /
            # rhs [128,2,N] -> out [128,N], contraction 256.
            # groups 0..NG-2 are DoubleRow over 256 samples; the last group's
            # two tiles run as normal-mode fp8 singles instead, so tile 14's
            # matmuls hide under tile 15's DMA and the serial tail after the
            # last tile lands is one 128-sample pass (plane-sliced APs)
            for g in range(NG):
                passes = [None] if g < NG - 1 else [0, 1]
                for pl in passes:
                    final = g == NG - 1 and pl == 1
                    for i, s0, s1, last_of_i in CHUNKS:
                        c0 = P * i
                        if pl is None:
                            nc.tensor.matmul(
                                pss[i][:, s0:s1],
                                lhsT=grps[g][:, :, c0 : c0 + P],
                                rhs=grps[g][:, :, c0 + s0 : c0 + s1],
                                start=(g == 0),
                                stop=False,
                                perf_mode=mybir.MatmulPerfMode.DoubleRow,
                            )
                        else:
                            nc.tensor.matmul(
                                pss[i][:, s0:s1],
                                lhsT=grps[g][:, pl, c0 : c0 + P],
                                rhs=grps[g][:, pl, c0 + s0 : c0 + s1],
                                start=False,
                                stop=final,
                            )
                        if final and last_of_i:
                            # final pass: copy out each finished row-block
                            # while the PE works on the remaining blocks;
                            # stream the packed triangle to DRAM in three
                            # slices (pairs of blocks) so stores overlap the
                            # remaining copies and the last store is small.
                            # A/B go on the idle sync ring; C on scalar
                            # (after the ACT copies retire).
                            _copy_out(nc, tri, pss, i)
                            if i == 1:
                                nc.sync.dma_start(
                                    out=out_sh[:, 0 : OFFS[1][1]],
                                    in_=tri[:, 0 : OFFS[1][1]],
                                )
                            elif i == 3:
                                nc.sync.dma_start(
                                    out=out_sh[:, OFFS[2][0] : OFFS[3][1]],
                                    in_=tri[:, OFFS[2][0] : OFFS[3][1]],
                                )
                            elif i == 5:
                                nc.scalar.dma_start(
                                    out=out_sh[:, OFFS[4][0] :],
                                    in_=tri[:, OFFS[4][0] :],
                                )
                if g < NG - 1:
                    # HAM pinning: the real fp8 groups only keep the PE at
                    # ~55-70% duty behind the load stream, which lets the HAM
                    # MID window re-throttle the clock mid-kernel.  A few
                    # junk matmuls into the ps0 padding columns after each
                    # group keep the PE dense enough to stay at K=8/8.
                    # start MUST be False: start=True clears the whole PSUM
                    # bank, which would wipe the live accumulation sharing
                    # the bank with the padding columns.
                    for _ in range(3):
                        nc.tensor.matmul(
                            ps0_full[:, D:1024],
                            lhsT=warm[:, 0:P],
                            rhs=warm[:, 0:256],
                            start=False,
                            stop=False,
                            skip_group_check=True,
                        )

    nc.finalize()  # Bacc: run reg-alloc + wait-legalization passes
    return nc


_NC_CACHE = None

# test-harness hooks (harness calls kernel() only; these stay defaults there)
RUN_KWARGS = {}
LAST_RESULTS = None


def _get_nc():
    global _NC_CACHE
    if _NC_CACHE is None:
        _NC_CACHE = _build()
    return _NC_CACHE


def kernel(x: np.ndarray) -> np.ndarray:
    global LAST_RESULTS
    x = np.ascontiguousarray(np.asarray(x, dtype=np.float32))
    assert x.shape == (NCORES, NSHARD, D)

    nc = _get_nc()
    in_maps = [{"x_shard": x[c]} for c in range(NCORES)]
    res = run_bass_kernel_spmd(
        nc, in_maps, core_ids=list(range(NCORES)), **RUN_KWARGS
    )
    LAST_RESULTS = res

    # gather/unshard: sum the raw partial triangles in f32, apply the affine
    packed = np.zeros((P, TRI_W), dtype=np.float32)
    for c in range(NCORES):
        packed += np.asarray(res.results[c]["out_shard"], dtype=np.float32)
    packed = packed * SCALE + BIAS
    packed = packed.reshape(P, NTRI, P).transpose(1, 0, 2)  # [21, 128, 128]

    out = np.empty((D, D), dtype=np.float32)
    for b, (i, j) in enumerate(TRI_BLOCKS):
        blk = packed[b]
        out[P * i : P * (i + 1), P * j : P * (j + 1)] = blk
        if j != i:
            out[P * j : P * (j + 1), P * i : P * (i + 1)] = blk.T
    return out


# revision 14
# speedup vs baseline: 1.0003x; 1.0003x over previous
"""Bass/Trainium2 kernel for nn_DecorrelationGradient.

Reference computation (KAPPA = 0.5):
    out = (1-k)*(gram - diag_ms) + k*(diag_ms - 1)
        = 0.5 * (X^T X / N) - 0.5          (diag terms cancel algebraically)

with X = x.reshape(N, d), N = 8*2048 = 16384, d = 768.

Strategy (data-parallel over the sample axis, 8 cores):
  - core c gets x[c] : [2048, 768] f32
  - per-core pipeline: HWDGE f32 loads (both rings) -> DVE cast to fp8 e4m3
    -> PE computes the upper-triangle blocks of the partial Gram x_c^T x_c
    with fp8 DoubleRow matmuls (256-sample contraction per matmul, ~2x the
    bf16 column rate; fp32 PSUM accumulation over 8 sample-groups).  The
    load stream runs at the per-core HBM limit and paces the kernel; the
    fp8 PE work hides entirely under it.
  - no on-device scale/bias: each core outputs its RAW partial Gram
    triangle packed [128, 2688] in bf16; the host gather sums the 8
    partials in f32 and applies out = sum * (0.5/N) - 0.5.  (fp8 inputs +
    bf16 partial output keep the overall rel err ~2-3e-4, well inside the
    1e-3 gate; a device-side ReduceScatter measures ~2.5x slower here
    because the collective serializes behind a cross-core launch-skew
    barrier plus ncfw trigger latency.)
"""

import numpy as np

import concourse.bacc as bacc
import concourse.bass as bass  # noqa: F401
import concourse.tile as tile
from concourse import mybir
from concourse.bass_utils import run_bass_kernel_spmd

P = 128
D = 768
NSHARD = 2048          # samples per core
KT = NSHARD // P       # 16 DMA tiles of 128 samples
NG = KT // 2           # 8 DoubleRow groups of 256 samples
NB = D // P            # 6 row/col blocks
NCORES = 8
NTOT = NCORES * NSHARD
SCALE = 0.5 / NTOT     # applied on host
BIAS = -0.5            # applied on host

# packed upper-triangle blocks (i, j) with j >= i, row-major in i
TRI_BLOCKS = [(i, j) for i in range(NB) for j in range(i, NB)]
NTRI = len(TRI_BLOCKS)          # 21
TRI_W = NTRI * P                # 2688 packed columns

# packed column range of row-block i
OFFS = []
_o = 0
for _i in range(NB):
    OFFS.append((_o, _o + (NB - _i) * P))
    _o = OFFS[-1][1]

# per-group matmul chunks (i, s0, s1, last_of_i), big blocks first so the
# widest copy-outs can start earliest in the final group, and so the
# LDWEIGHTS stream (one per block, overlapped via the background weight
# buffer) keeps pace with the matmul stream
CHUNKS = []
for _i in range(NB):
    w = D - P * _i
    if w > 512:
        CHUNKS.append((_i, 0, 512, False))
        CHUNKS.append((_i, 512, w, True))
    else:
        CHUNKS.append((_i, 0, w, True))


def _copy_out(nc, tri, pss, i):
    """Plain PSUM->SBUF bf16 copy of row-block i (scale/bias is host-side).
    Blocks alternate between the ACT and DVE engines so consecutive blocks
    copy in parallel as their accumulators finish (GPSIMD has no PSUM port)."""
    o0, o1 = OFFS[i]
    if i % 2 == 0:
        nc.scalar.activation(
            out=tri[:, o0:o1],
            in_=pss[i][:, 0 : o1 - o0],
            func=mybir.ActivationFunctionType.Copy,
        )
    else:
        nc.vector.tensor_copy(out=tri[:, o0:o1], in_=pss[i][:, 0 : o1 - o0])


def _build():
    nc = bacc.Bacc(num_devices=NCORES)

    x_sh = nc.dram_tensor(
        "x_shard", [NSHARD, D], mybir.dt.float32, kind="ExternalInput"
    )
    out_sh = nc.dram_tensor(
        "out_shard", [P, TRI_W], mybir.dt.bfloat16, kind="ExternalOutput"
    )

    f32 = mybir.dt.float32
    bf16 = mybir.dt.bfloat16
    fp8 = mybir.dt.float8e4

    with tile.TileContext(nc) as tc:
        with (
            tc.tile_pool(name="xp", bufs=KT) as xpool,
            tc.tile_pool(name="gp", bufs=NG) as gpool,
            tc.tile_pool(name="ps", bufs=1, space="PSUM") as pspool,
            tc.tile_pool(name="acc", bufs=1) as accpool,
        ):
            # HAM warmup tile: first thing in program order so the junk
            # matmuls fill the PE pipe while the first x tiles stream in
            warm = gpool.tile([P, 512], bf16, tag="warm", name="warm")
            nc.gpsimd.memset(warm[:], 0.0)

            # pipeline per 128-sample tile: HWDGE f32 DMA -> DVE cast to
            # fp8 plane (k%2) of group k//2.  loads alternate between the
            # two physical HWDGE rings (SP / ACT)
            grps = []
            for g in range(NG):
                grps.append(gpool.tile([P, 2, D], fp8, tag="gx", name=f"gx{g}"))
            for k in range(KT):
                stage = xpool.tile([P, D], f32, tag="xs", name=f"xs{k}")
                if k < KT - 2:
                    dma_eng = nc.sync if k % 2 == 0 else nc.scalar
                    dma_eng.dma_start(
                        out=stage[:], in_=x_sh[k * P : (k + 1) * P, :]
                    )
                    nc.vector.tensor_copy(
                        out=grps[k // 2][:, k % 2, :], in_=stage[:]
                    )
                else:
                    # last pair: split each tile into column halves on both
                    # rings and cast per half, so only ~half a cast remains
                    # after the final DMA lands (shorter serial tail)
                    h = D // 2
                    nc.sync.dma_start(
                        out=stage[:, 0:h], in_=x_sh[k * P : (k + 1) * P, 0:h]
                    )
                    nc.scalar.dma_start(
                        out=stage[:, h:D], in_=x_sh[k * P : (k + 1) * P, h:D]
                    )
                    nc.vector.tensor_copy(
                        out=grps[k // 2][:, k % 2, 0:h], in_=stage[:, 0:h]
                    )
                    nc.vector.tensor_copy(
                        out=grps[k // 2][:, k % 2, h:D], in_=stage[:, h:D]
                    )

            tri = accpool.tile([P, TRI_W], bf16)  # packed raw partial triangle

            # psum accumulators, one per row-block; exactly 8 PSUM banks.
            # row-block i covers G[i-block, j-blocks j>=i] = cols 128*i..768.
            # ps0 is padded to its full 2-bank allocation; the padding columns
            # [768:1024] are a scratch target for the HAM-pinning junk matmuls
            # (per-element has_written bits keep them out of the real data).
            ps0_full = pspool.tile([P, 1024], f32, tag="ps0", name="ps0")
            pss = [ps0_full]
            for i in range(1, NB):
                pss.append(
                    pspool.tile([P, D - P * i], f32, tag=f"ps{i}", name=f"ps{i}")
                )

            # HAM warmup: junk matmuls on the zeroed tile keep the PE busy
            # until the first real tile lands.  The fp8 groups leave the PE
            # at ~55% duty behind the load stream, which is not dense enough
            # to flip the HAM activity window on its own — so run ~3.8us of
            # dense junk up front to un-throttle the clock before group 0.
            # Junk goes to pss[0]; the real g=0 matmul start=True resets it.
            for w in range(9):
                nc.tensor.matmul(
                    pss[0][:, 0:512],
                    lhsT=warm[:, 0:P],
                    rhs=warm[:],
                    start=True,
                    stop=True,
                )

            # group-outer / chunk-inner: each 256-sample fp8 group is fully
            # consumed right after its two DMA+cast tiles land, so the PE
            # overlaps the load stream.  DoubleRow: lhsT [128,2,128] /
            # rhs [128,2,N] -> out [128,N], contraction 256.
            # groups 0..NG-2 are DoubleRow over 256 samples; the last group's
            # two tiles run as normal-mode fp8 singles instead, so tile 14's
            # matmuls hide under tile 15's DMA and the serial tail after the
            # last tile lands is one 128-sample pass (plane-sliced APs)
            for g in range(NG):
                final = g == NG - 1
                for i, s0, s1, last_of_i in CHUNKS:
                    c0 = P * i
                    nc.tensor.matmul(
                        pss[i][:, s0:s1],
                        lhsT=grps[g][:, :, c0 : c0 + P],
                        rhs=grps[g][:, :, c0 + s0 : c0 + s1],
                        start=(g == 0),
                        stop=final,
                        perf_mode=mybir.MatmulPerfMode.DoubleRow,
                    )
                    if final and last_of_i:
                        # final group: copy out each finished row-block while
                        # the PE works on the remaining blocks; stream the
                        # packed triangle to DRAM in three slices (pairs of
                        # blocks) so stores overlap the remaining copies and
                        # the last store is small.  A/B go on the idle sync
                        # ring; C on scalar (after the ACT copies retire).
                        _copy_out(nc, tri, pss, i)
                        if i == 1:
                            nc.sync.dma_start(
                                out=out_sh[:, 0 : OFFS[1][1]],
                                in_=tri[:, 0 : OFFS[1][1]],
                            )
                        elif i == 3:
                            nc.sync.dma_start(
                                out=out_sh[:, OFFS[2][0] : OFFS[3][1]],
                                in_=tri[:, OFFS[2][0] : OFFS[3][1]],
                            )
                        elif i == 5:
                            nc.scalar.dma_start(
                                out=out_sh[:, OFFS[4][0] :],
                                in_=tri[:, OFFS[4][0] :],
                            )
                if g < NG - 2:
                    # HAM pinning: the real fp8 groups only keep the PE at
                    # ~55-70% duty behind the load stream, which lets the HAM
                    # MID window re-throttle the clock mid-kernel.  A few
                    # junk matmuls into the ps0 padding columns after each
                    # group keep the PE dense enough to stay at K=8/8.
                    # start MUST be False: start=True clears the whole PSUM
                    # bank, which would wipe the live accumulation sharing
                    # the bank with the padding columns.
                    for _ in range(3):
                        nc.tensor.matmul(
                            ps0_full[:, D:1024],
                            lhsT=warm[:, 0:P],
                            rhs=warm[:, 0:256],
                            start=False,
                            stop=False,
                            skip_group_check=True,
                        )

    nc.finalize()  # Bacc: run reg-alloc + wait-legalization passes
    return nc


_NC_CACHE = None

# test-harness hooks (harness calls kernel() only; these stay defaults there)
RUN_KWARGS = {}
LAST_RESULTS = None


def _get_nc():
    global _NC_CACHE
    if _NC_CACHE is None:
        _NC_CACHE = _build()
    return _NC_CACHE


def kernel(x: np.ndarray) -> np.ndarray:
    global LAST_RESULTS
    x = np.ascontiguousarray(np.asarray(x, dtype=np.float32))
    assert x.shape == (NCORES, NSHARD, D)

    nc = _get_nc()
    in_maps = [{"x_shard": x[c]} for c in range(NCORES)]
    res = run_bass_kernel_spmd(
        nc, in_maps, core_ids=list(range(NCORES)), **RUN_KWARGS
    )
    LAST_RESULTS = res

    # gather/unshard: sum the raw partial triangles in f32, apply the affine
    packed = np.zeros((P, TRI_W), dtype=np.float32)
    for c in range(NCORES):
        packed += np.asarray(res.results[c]["out_shard"], dtype=np.float32)
    packed = packed * SCALE + BIAS
    packed = packed.reshape(P, NTRI, P).transpose(1, 0, 2)  # [21, 128, 128]

    out = np.empty((D, D), dtype=np.float32)
    for b, (i, j) in enumerate(TRI_BLOCKS):
        blk = packed[b]
        out[P * i : P * (i + 1), P * j : P * (j + 1)] = blk
        if j != i:
            out[P * j : P * (j + 1), P * i : P * (i + 1)] = blk.T
    return out


# revision 17
# speedup vs baseline: 1.0235x; 1.0233x over previous
"""Bass/Trainium2 kernel for nn_DecorrelationGradient.

Reference computation (KAPPA = 0.5):
    out = (1-k)*(gram - diag_ms) + k*(diag_ms - 1)
        = 0.5 * (X^T X / N) - 0.5          (diag terms cancel algebraically)

with X = x.reshape(N, d), N = 8*2048 = 16384, d = 768.

Strategy (data-parallel over the sample axis, 8 cores):
  - core c gets x[c] : [2048, 768] f32
  - per-core pipeline: HWDGE f32 loads (both rings) -> DVE cast to fp8 e4m3
    -> PE computes the upper-triangle blocks of the partial Gram x_c^T x_c
    with fp8 DoubleRow matmuls (256-sample contraction per matmul, ~2x the
    bf16 column rate; fp32 PSUM accumulation over 8 sample-groups).  The
    load stream runs at the per-core HBM limit and paces the kernel; the
    fp8 PE work hides entirely under it.
  - no on-device scale/bias: each core outputs its RAW partial Gram
    triangle packed [128, 2688] in bf16; the host gather sums the 8
    partials in f32 and applies out = sum * (0.5/N) - 0.5.  (fp8 inputs +
    bf16 partial output keep the overall rel err ~2-3e-4, well inside the
    1e-3 gate; a device-side ReduceScatter measures ~2.5x slower here
    because the collective serializes behind a cross-core launch-skew
    barrier plus ncfw trigger latency.)
"""

import numpy as np

import concourse.bacc as bacc
import concourse.bass as bass  # noqa: F401
import concourse.tile as tile
from concourse import mybir
from concourse.bass_utils import run_bass_kernel_spmd

P = 128
D = 768
NSHARD = 2048          # samples per core
KT = NSHARD // P       # 16 DMA tiles of 128 samples
NG = KT // 2           # 8 DoubleRow groups of 256 samples
NB = D // P            # 6 row/col blocks
NCORES = 8
NTOT = NCORES * NSHARD
SCALE = 0.5 / NTOT     # applied on host
BIAS = -0.5            # applied on host

# packed upper-triangle blocks (i, j) with j >= i, row-major in i
TRI_BLOCKS = [(i, j) for i in range(NB) for j in range(i, NB)]
NTRI = len(TRI_BLOCKS)          # 21
TRI_W = NTRI * P                # 2688 packed columns

# packed column range of row-block i
OFFS = []
_o = 0
for _i in range(NB):
    OFFS.append((_o, _o + (NB - _i) * P))
    _o = OFFS[-1][1]

# per-group matmul chunks (i, s0, s1, last_of_i), big blocks first so the
# widest copy-outs can start earliest in the final group, and so the
# LDWEIGHTS stream (one per block, overlapped via the background weight
# buffer) keeps pace with the matmul stream
CHUNKS = []
for _i in range(NB):
    w = D - P * _i
    if w > 512:
        CHUNKS.append((_i, 0, 512, False))
        CHUNKS.append((_i, 512, w, True))
    else:
        CHUNKS.append((_i, 0, w, True))


# if True, issue all whole-tile input DMAs on the sync ring only (strictly
# sequential HBM read stream); stores stay split across rings
SINGLE_RING = True


def _copy_out(nc, tri, pss, i):
    """Plain PSUM->SBUF bf16 copy of row-block i (scale/bias is host-side).
    Blocks alternate between the ACT and DVE engines so consecutive blocks
    copy in parallel as their accumulators finish (GPSIMD has no PSUM port)."""
    o0, o1 = OFFS[i]
    if i % 2 == 0:
        nc.scalar.activation(
            out=tri[:, o0:o1],
            in_=pss[i][:, 0 : o1 - o0],
            func=mybir.ActivationFunctionType.Copy,
        )
    else:
        nc.vector.tensor_copy(out=tri[:, o0:o1], in_=pss[i][:, 0 : o1 - o0])


def _build():
    nc = bacc.Bacc(num_devices=NCORES)

    x_sh = nc.dram_tensor(
        "x_shard", [NSHARD, D], mybir.dt.float32, kind="ExternalInput"
    )
    out_sh = nc.dram_tensor(
        "out_shard", [P, TRI_W], mybir.dt.bfloat16, kind="ExternalOutput"
    )

    f32 = mybir.dt.float32
    bf16 = mybir.dt.bfloat16
    fp8 = mybir.dt.float8e4

    with tile.TileContext(nc) as tc:
        with (
            tc.tile_pool(name="xp", bufs=KT) as xpool,
            tc.tile_pool(name="gp", bufs=NG) as gpool,
            tc.tile_pool(name="ps", bufs=1, space="PSUM") as pspool,
            tc.tile_pool(name="acc", bufs=1) as accpool,
        ):
            # HAM warmup tile: first thing in program order so the junk
            # matmuls fill the PE pipe while the first x tiles stream in
            warm = gpool.tile([P, 512], bf16, tag="warm", name="warm")
            nc.gpsimd.memset(warm[:], 0.0)

            # pipeline per 128-sample tile: HWDGE f32 DMA -> DVE cast to
            # fp8 plane (k%2) of group k//2.  loads alternate between the
            # two physical HWDGE rings (SP / ACT)
            grps = []
            for g in range(NG):
                grps.append(gpool.tile([P, 2, D], fp8, tag="gx", name=f"gx{g}"))
            for k in range(KT):
                stage = xpool.tile([P, D], f32, tag="xs", name=f"xs{k}")
                if k < KT - 2:
                    if SINGLE_RING:
                        dma_eng = nc.sync
                    else:
                        dma_eng = nc.sync if k % 2 == 0 else nc.scalar
                    dma_eng.dma_start(
                        out=stage[:], in_=x_sh[k * P : (k + 1) * P, :]
                    )
                    nc.vector.tensor_copy(
                        out=grps[k // 2][:, k % 2, :], in_=stage[:]
                    )
                else:
                    # last pair: split each tile into column halves on both
                    # rings and cast per half, so only ~half a cast remains
                    # after the final DMA lands (shorter serial tail)
                    h = D // 2
                    nc.sync.dma_start(
                        out=stage[:, 0:h], in_=x_sh[k * P : (k + 1) * P, 0:h]
                    )
                    nc.scalar.dma_start(
                        out=stage[:, h:D], in_=x_sh[k * P : (k + 1) * P, h:D]
                    )
                    nc.vector.tensor_copy(
                        out=grps[k // 2][:, k % 2, 0:h], in_=stage[:, 0:h]
                    )
                    nc.vector.tensor_copy(
                        out=grps[k // 2][:, k % 2, h:D], in_=stage[:, h:D]
                    )

            tri = accpool.tile([P, TRI_W], bf16)  # packed raw partial triangle

            # psum accumulators, one per row-block; exactly 8 PSUM banks.
            # row-block i covers G[i-block, j-blocks j>=i] = cols 128*i..768.
            # ps0 is padded to its full 2-bank allocation; the padding columns
            # [768:1024] are a scratch target for the HAM-pinning junk matmuls
            # (per-element has_written bits keep them out of the real data).
            ps0_full = pspool.tile([P, 1024], f32, tag="ps0", name="ps0")
            pss = [ps0_full]
            for i in range(1, NB):
                pss.append(
                    pspool.tile([P, D - P * i], f32, tag=f"ps{i}", name=f"ps{i}")
                )

            # HAM warmup: junk matmuls on the zeroed tile keep the PE busy
            # until the first real tile lands.  The fp8 groups leave the PE
            # at ~55% duty behind the load stream, which is not dense enough
            # to flip the HAM activity window on its own — so run ~3.8us of
            # dense junk up front to un-throttle the clock before group 0.
            # Junk goes to pss[0]; the real g=0 matmul start=True resets it.
            for w in range(9):
                nc.tensor.matmul(
                    pss[0][:, 0:512],
                    lhsT=warm[:, 0:P],
                    rhs=warm[:],
                    start=True,
                    stop=True,
                )

            # group-outer / chunk-inner: each 256-sample fp8 group is fully
            # consumed right after its two DMA+cast tiles land, so the PE
            # overlaps the load stream.  DoubleRow: lhsT [128,2,128] /
            # rhs [128,2,N] -> out [128,N], contraction 256.
            # groups 0..NG-2 are DoubleRow over 256 samples; the last group's
            # two tiles run as normal-mode fp8 singles instead, so tile 14's
            # matmuls hide under tile 15's DMA and the serial tail after the
            # last tile lands is one 128-sample pass (plane-sliced APs)
            for g in range(NG):
                final = g == NG - 1
                for i, s0, s1, last_of_i in CHUNKS:
                    c0 = P * i
                    nc.tensor.matmul(
                        pss[i][:, s0:s1],
                        lhsT=grps[g][:, :, c0 : c0 + P],
                        rhs=grps[g][:, :, c0 + s0 : c0 + s1],
                        start=(g == 0),
                        stop=final,
                        perf_mode=mybir.MatmulPerfMode.DoubleRow,
                    )
                    if final and last_of_i:
                        # final group: copy out each finished row-block while
                        # the PE works on the remaining blocks; stream the
                        # packed triangle to DRAM in three slices (pairs of
                        # blocks) so stores overlap the remaining copies and
                        # the last store is small.  A/B go on the idle sync
                        # ring; C on scalar (after the ACT copies retire).
                        _copy_out(nc, tri, pss, i)
                        if i == 1:
                            nc.sync.dma_start(
                                out=out_sh[:, 0 : OFFS[1][1]],
                                in_=tri[:, 0 : OFFS[1][1]],
                            )
                        elif i == 3:
                            nc.sync.dma_start(
                                out=out_sh[:, OFFS[2][0] : OFFS[3][1]],
                                in_=tri[:, OFFS[2][0] : OFFS[3][1]],
                            )
                        elif i == 5:
                            nc.scalar.dma_start(
                                out=out_sh[:, OFFS[4][0] :],
                                in_=tri[:, OFFS[4][0] :],
                            )
                if g < NG - 2:
                    # HAM pinning: the real fp8 groups only keep the PE at
                    # ~55-70% duty behind the load stream, which lets the HAM
                    # MID window re-throttle the clock mid-kernel.  A few
                    # junk matmuls into the ps0 padding columns after each
                    # group keep the PE dense enough to stay at K=8/8.
                    # start MUST be False: start=True clears the whole PSUM
                    # bank, which would wipe the live accumulation sharing
                    # the bank with the padding columns.
                    for _ in range(3):
                        nc.tensor.matmul(
                            ps0_full[:, D:1024],
                            lhsT=warm[:, 0:P],
                            rhs=warm[:, 0:256],
                            start=False,
                            stop=False,
                            skip_group_check=True,
                        )

    nc.finalize()  # Bacc: run reg-alloc + wait-legalization passes
    return nc


_NC_CACHE = None

# test-harness hooks (harness calls kernel() only; these stay defaults there)
RUN_KWARGS = {}
LAST_RESULTS = None


def _get_nc():
    global _NC_CACHE
    if _NC_CACHE is None:
        _NC_CACHE = _build()
    return _NC_CACHE


def kernel(x: np.ndarray) -> np.ndarray:
    global LAST_RESULTS
    x = np.ascontiguousarray(np.asarray(x, dtype=np.float32))
    assert x.shape == (NCORES, NSHARD, D)

    nc = _get_nc()
    in_maps = [{"x_shard": x[c]} for c in range(NCORES)]
    res = run_bass_kernel_spmd(
        nc, in_maps, core_ids=list(range(NCORES)), **RUN_KWARGS
    )
    LAST_RESULTS = res

    # gather/unshard: sum the raw partial triangles in f32, apply the affine
    packed = np.zeros((P, TRI_W), dtype=np.float32)
    for c in range(NCORES):
        packed += np.asarray(res.results[c]["out_shard"], dtype=np.float32)
    packed = packed * SCALE + BIAS
    packed = packed.reshape(P, NTRI, P).transpose(1, 0, 2)  # [21, 128, 128]

    out = np.empty((D, D), dtype=np.float32)
    for b, (i, j) in enumerate(TRI_BLOCKS):
        blk = packed[b]
        out[P * i : P * (i + 1), P * j : P * (j + 1)] = blk
        if j != i:
            out[P * j : P * (j + 1), P * i : P * (i + 1)] = blk.T
    return out


# revision 18
# speedup vs baseline: 1.0588x; 1.0344x over previous
"""Bass/Trainium2 kernel for nn_DecorrelationGradient.

Reference computation (KAPPA = 0.5):
    out = (1-k)*(gram - diag_ms) + k*(diag_ms - 1)
        = 0.5 * (X^T X / N) - 0.5          (diag terms cancel algebraically)

with X = x.reshape(N, d), N = 8*2048 = 16384, d = 768.

Strategy (data-parallel over the sample axis, 8 cores):
  - core c gets x[c] : [2048, 768] f32
  - per-core pipeline: HWDGE f32 loads (both rings) -> DVE cast to fp8 e4m3
    -> PE computes the upper-triangle blocks of the partial Gram x_c^T x_c
    with fp8 DoubleRow matmuls (256-sample contraction per matmul, ~2x the
    bf16 column rate; fp32 PSUM accumulation over 8 sample-groups).  The
    load stream runs at the per-core HBM limit and paces the kernel; the
    fp8 PE work hides entirely under it.
  - no on-device scale/bias: each core outputs its RAW partial Gram
    triangle packed [128, 2688] in bf16; the host gather sums the 8
    partials in f32 and applies out = sum * (0.5/N) - 0.5.  (fp8 inputs +
    bf16 partial output keep the overall rel err ~2-3e-4, well inside the
    1e-3 gate; a device-side ReduceScatter measures ~2.5x slower here
    because the collective serializes behind a cross-core launch-skew
    barrier plus ncfw trigger latency.)
"""

import numpy as np

import concourse.bacc as bacc
import concourse.bass as bass  # noqa: F401
import concourse.tile as tile
from concourse import mybir
from concourse.bass_utils import run_bass_kernel_spmd

P = 128
D = 768
NSHARD = 2048          # samples per core
KT = NSHARD // P       # 16 DMA tiles of 128 samples
NG = KT // 2           # 8 DoubleRow groups of 256 samples
NB = D // P            # 6 row/col blocks
NCORES = 8
NTOT = NCORES * NSHARD
SCALE = 0.5 / NTOT     # applied on host
BIAS = -0.5            # applied on host

# packed upper-triangle blocks (i, j) with j >= i, row-major in i
TRI_BLOCKS = [(i, j) for i in range(NB) for j in range(i, NB)]
NTRI = len(TRI_BLOCKS)          # 21
TRI_W = NTRI * P                # 2688 packed columns

# packed column range of row-block i
OFFS = []
_o = 0
for _i in range(NB):
    OFFS.append((_o, _o + (NB - _i) * P))
    _o = OFFS[-1][1]

# per-group matmul chunks (i, s0, s1, last_of_i), big blocks first so the
# widest copy-outs can start earliest in the final group, and so the
# LDWEIGHTS stream (one per block, overlapped via the background weight
# buffer) keeps pace with the matmul stream
CHUNKS = []
for _i in range(NB):
    w = D - P * _i
    if w > 512:
        CHUNKS.append((_i, 0, 512, False))
        CHUNKS.append((_i, 512, w, True))
    else:
        CHUNKS.append((_i, 0, w, True))


# if True, issue all whole-tile input DMAs on the sync ring only (strictly
# sequential HBM read stream); stores stay split across rings
SINGLE_RING = False


def _copy_out(nc, tri, pss, i):
    """Plain PSUM->SBUF bf16 copy of row-block i (scale/bias is host-side).
    Blocks alternate between the ACT and DVE engines so consecutive blocks
    copy in parallel as their accumulators finish (GPSIMD has no PSUM port)."""
    o0, o1 = OFFS[i]
    if i % 2 == 0:
        nc.scalar.activation(
            out=tri[:, o0:o1],
            in_=pss[i][:, 0 : o1 - o0],
            func=mybir.ActivationFunctionType.Copy,
        )
    else:
        nc.vector.tensor_copy(out=tri[:, o0:o1], in_=pss[i][:, 0 : o1 - o0])


def _build():
    nc = bacc.Bacc(num_devices=NCORES)

    x_sh = nc.dram_tensor(
        "x_shard", [NSHARD, D], mybir.dt.float32, kind="ExternalInput"
    )
    out_sh = nc.dram_tensor(
        "out_shard", [P, TRI_W], mybir.dt.bfloat16, kind="ExternalOutput"
    )

    f32 = mybir.dt.float32
    bf16 = mybir.dt.bfloat16
    fp8 = mybir.dt.float8e4

    with tile.TileContext(nc) as tc:
        with (
            tc.tile_pool(name="xp", bufs=KT) as xpool,
            tc.tile_pool(name="gp", bufs=NG) as gpool,
            tc.tile_pool(name="ps", bufs=1, space="PSUM") as pspool,
            tc.tile_pool(name="acc", bufs=1) as accpool,
        ):
            # HAM warmup tile: first thing in program order so the junk
            # matmuls fill the PE pipe while the first x tiles stream in
            warm = gpool.tile([P, 512], bf16, tag="warm", name="warm")
            nc.gpsimd.memset(warm[:], 0.0)

            # pipeline per 128-sample tile: HWDGE f32 DMA -> DVE cast to
            # fp8 plane (k%2) of group k//2.  loads alternate between the
            # two physical HWDGE rings (SP / ACT)
            grps = []
            for g in range(NG):
                grps.append(gpool.tile([P, 2, D], fp8, tag="gx", name=f"gx{g}"))
            for k in range(KT):
                stage = xpool.tile([P, D], f32, tag="xs", name=f"xs{k}")
                if k < KT - 2:
                    if SINGLE_RING:
                        dma_eng = nc.sync
                    else:
                        dma_eng = nc.sync if k % 2 == 0 else nc.scalar
                    dma_eng.dma_start(
                        out=stage[:], in_=x_sh[k * P : (k + 1) * P, :]
                    )
                    nc.vector.tensor_copy(
                        out=grps[k // 2][:, k % 2, :], in_=stage[:]
                    )
                else:
                    # last pair: split each tile into column halves on both
                    # rings and cast per half, so only ~half a cast remains
                    # after the final DMA lands (shorter serial tail)
                    h = D // 2
                    nc.sync.dma_start(
                        out=stage[:, 0:h], in_=x_sh[k * P : (k + 1) * P, 0:h]
                    )
                    nc.scalar.dma_start(
                        out=stage[:, h:D], in_=x_sh[k * P : (k + 1) * P, h:D]
                    )
                    nc.vector.tensor_copy(
                        out=grps[k // 2][:, k % 2, 0:h], in_=stage[:, 0:h]
                    )
                    nc.vector.tensor_copy(
                        out=grps[k // 2][:, k % 2, h:D], in_=stage[:, h:D]
                    )

            tri = accpool.tile([P, TRI_W], bf16)  # packed raw partial triangle

            # psum accumulators, one per row-block; exactly 8 PSUM banks.
            # row-block i covers G[i-block, j-blocks j>=i] = cols 128*i..768.
            # ps0 is padded to its full 2-bank allocation; the padding columns
            # [768:1024] are a scratch target for the HAM-pinning junk matmuls
            # (per-element has_written bits keep them out of the real data).
            ps0_full = pspool.tile([P, 1024], f32, tag="ps0", name="ps0")
            pss = [ps0_full]
            for i in range(1, NB):
                pss.append(
                    pspool.tile([P, D - P * i], f32, tag=f"ps{i}", name=f"ps{i}")
                )

            # HAM warmup: junk matmuls on the zeroed tile keep the PE busy
            # until the first real tile lands.  The fp8 groups leave the PE
            # at ~55% duty behind the load stream, which is not dense enough
            # to flip the HAM activity window on its own — so run ~3.8us of
            # dense junk up front to un-throttle the clock before group 0.
            # Junk goes to pss[0]; the real g=0 matmul start=True resets it.
            for w in range(9):
                nc.tensor.matmul(
                    pss[0][:, 0:512],
                    lhsT=warm[:, 0:P],
                    rhs=warm[:],
                    start=True,
                    stop=True,
                )

            # group-outer / chunk-inner: each 256-sample fp8 group is fully
            # consumed right after its two DMA+cast tiles land, so the PE
            # overlaps the load stream.  DoubleRow: lhsT [128,2,128] /
            # rhs [128,2,N] -> out [128,N], contraction 256.
            # groups 0..NG-2 are DoubleRow over 256 samples; the last group's
            # two tiles run as normal-mode fp8 singles instead, so tile 14's
            # matmuls hide under tile 15's DMA and the serial tail after the
            # last tile lands is one 128-sample pass (plane-sliced APs)
            for g in range(NG):
                final = g == NG - 1
                for i, s0, s1, last_of_i in CHUNKS:
                    c0 = P * i
                    nc.tensor.matmul(
                        pss[i][:, s0:s1],
                        lhsT=grps[g][:, :, c0 : c0 + P],
                        rhs=grps[g][:, :, c0 + s0 : c0 + s1],
                        start=(g == 0),
                        stop=final,
                        perf_mode=mybir.MatmulPerfMode.DoubleRow,
                    )
                    if final and last_of_i:
                        # final group: copy out each finished row-block while
                        # the PE works on the remaining blocks; stream the
                        # packed triangle to DRAM in three slices (pairs of
                        # blocks) so stores overlap the remaining copies and
                        # the last store is small.  A/B go on the idle sync
                        # ring; C on scalar (after the ACT copies retire).
                        _copy_out(nc, tri, pss, i)
                        if i == 1:
                            nc.sync.dma_start(
                                out=out_sh[:, 0 : OFFS[1][1]],
                                in_=tri[:, 0 : OFFS[1][1]],
                            )
                        elif i == 3:
                            nc.sync.dma_start(
                                out=out_sh[:, OFFS[2][0] : OFFS[3][1]],
                                in_=tri[:, OFFS[2][0] : OFFS[3][1]],
                            )
                        elif i == 5:
                            nc.scalar.dma_start(
                                out=out_sh[:, OFFS[4][0] :],
                                in_=tri[:, OFFS[4][0] :],
                            )
                if g < NG - 2:
                    # HAM pinning: the real fp8 groups only keep the PE at
                    # ~55-70% duty behind the load stream, which lets the HAM
                    # MID window re-throttle the clock mid-kernel.  A few
                    # junk matmuls into the ps0 padding columns after each
                    # group keep the PE dense enough to stay at K=8/8.
                    # start MUST be False: start=True clears the whole PSUM
                    # bank, which would wipe the live accumulation sharing
                    # the bank with the padding columns.
                    for _ in range(3):
                        nc.tensor.matmul(
                            ps0_full[:, D:1024],
                            lhsT=warm[:, 0:P],
                            rhs=warm[:, 0:256],
                            start=False,
                            stop=False,
                            skip_group_check=True,
                        )

    nc.finalize()  # Bacc: run reg-alloc + wait-legalization passes
    return nc


_NC_CACHE = None

# test-harness hooks (harness calls kernel() only; these stay defaults there)
RUN_KWARGS = {}
LAST_RESULTS = None


def _get_nc():
    global _NC_CACHE
    if _NC_CACHE is None:
        _NC_CACHE = _build()
    return _NC_CACHE


def kernel(x: np.ndarray) -> np.ndarray:
    global LAST_RESULTS
    x = np.ascontiguousarray(np.asarray(x, dtype=np.float32))
    assert x.shape == (NCORES, NSHARD, D)

    nc = _get_nc()
    in_maps = [{"x_shard": x[c]} for c in range(NCORES)]
    res = run_bass_kernel_spmd(
        nc, in_maps, core_ids=list(range(NCORES)), **RUN_KWARGS
    )
    LAST_RESULTS = res

    # gather/unshard: sum the raw partial triangles in f32, apply the affine
    packed = np.zeros((P, TRI_W), dtype=np.float32)
    for c in range(NCORES):
        packed += np.asarray(res.results[c]["out_shard"], dtype=np.float32)
    packed = packed * SCALE + BIAS
    packed = packed.reshape(P, NTRI, P).transpose(1, 0, 2)  # [21, 128, 128]

    out = np.empty((D, D), dtype=np.float32)
    for b, (i, j) in enumerate(TRI_BLOCKS):
        blk = packed[b]
        out[P * i : P * (i + 1), P * j : P * (j + 1)] = blk
        if j != i:
            out[P * j : P * (j + 1), P * i : P * (i + 1)] = blk.T
    return out
